# revision 9
# baseline (speedup 1.0000x reference)
"""Trainium2 Bass kernel for LocallyDirected1D (gnn_message_passing).

Computation: out[b, j] = sum_{e in [o[j], o[j+1])} x[b, e] * k[e]  (+ bias[j])
where o = CSR offsets of the sorted mask_col, mask_row == arange(E).

Strategy:
  * Data-parallel over batch: core c handles batch rows [8c, 8c+8).
  * Host builds a padded-ELL layout: output columns sorted by segment
    length, grouped into superblocks of 1024 columns (8 groups x 128
    partitions) padded to the superblock max length P.  Padding slots of
    the weight array are zero, so the gathered x can hold garbage there.
    The x slab and the k slab of a superblock are packed into ONE
    contiguous HBM region so a single DMA (= a single semaphore wait)
    brings in both.
  * Device streams the dense slabs: y = x*k on VectorE, segment sums via
    tensor_reduce over the innermost (padded-length) axis, results DMAed
    back and un-permuted on host.
"""

import numpy as np

import concourse.bass as bass
import concourse.mybir as mybir
from concourse.tile import TileContext
from concourse.bass_utils import run_bass_kernel_spmd

B = 64
E = 1_000_000
NOUT = 20_000
NCORES = 8
BPC = B // NCORES          # batch rows per core
CPB = 128                  # columns per block (partition dim)
G = 8                      # blocks per superblock
SBC = CPB * G              # 1024 columns per superblock
NSB = (NOUT + SBC - 1) // SBC   # 20 superblocks
NPAD = NSB * SBC - NOUT    # dummy (zero-length) columns, placed first
ROWW = BPC * G + G         # free elems per partition per unit P (x then k)

F32 = mybir.dt.float32


def _plan(mask_col: np.ndarray):
    """CSR offsets -> length-sorted padded-ELL plan."""
    o = np.searchsorted(mask_col, np.arange(NOUT + 1)).astype(np.int64)
    lens = np.diff(o).astype(np.int64)
    perm = np.argsort(lens, kind="stable").astype(np.int64)
    lens_s = np.concatenate([np.zeros(NPAD, np.int64), lens[perm]])
    starts_s = np.concatenate([np.zeros(NPAD, np.int64), o[:-1][perm]])
    P = lens_s.reshape(NSB, SBC).max(axis=1)
    P = np.maximum(P, 1).astype(np.int64)
    return perm, lens_s, starts_s, P


def _build_program(P):
    nc = bass.Bass()
    off = np.concatenate([[0], np.cumsum(128 * ROWW * P)]).astype(np.int64)
    xk_d = nc.dram_tensor("xkell", [int(off[-1])], F32, kind="ExternalInput")
    # one output tensor per superblock: a single shared output tensor makes
    # Tile serialize the store DMAs (tensor-granularity WAW), which puts a
    # second sync-wait on each store — walrus allows only one per instruction
    o_ds = [
        nc.dram_tensor(f"oseg{sb:02d}", [128 * BPC * G], F32,
                       kind="ExternalOutput")
        for sb in range(NSB)
    ]

    with TileContext(nc) as tc:
        with (
            tc.tile_pool(name="xp", bufs=3) as xp,
            tc.tile_pool(name="op", bufs=NSB) as op_,
        ):
            for sb in range(NSB):
                Ps = int(P[sb])
                QF = G * Ps            # free elems per batch row (and k width)
                XF = BPC * QF          # x portion width
                t = xp.tile([128, ROWW * Ps], F32, tag="x")
                nc.sync.dma_start(
                    t[:],
                    xk_d[int(off[sb]):int(off[sb + 1])].rearrange(
                        "(j f) -> j f", j=128),
                )
                for b in range(BPC):
                    nc.vector.tensor_tensor(
                        t[:, b * QF:(b + 1) * QF],
                        t[:, b * QF:(b + 1) * QF],
                        t[:, XF:XF + QF],
                        mybir.AluOpType.mult,
                    )
                ot = op_.tile([128, BPC * G], F32, tag="o")
                nc.vector.tensor_reduce(
                    ot[:],
                    t[:, 0:XF].rearrange("j (q p) -> j q p", p=Ps),
                    axis=mybir.AxisListType.X,
                    op=mybir.AluOpType.add,
                )
                nc.sync.dma_start(
                    o_ds[sb][:].rearrange("(j f) -> j f", j=128),
                    ot[:],
                )
    return nc, off


def _split_multi_waits(nc):
    """walrus allows at most one sync-wait per engine instruction; hoist
    extra waits into standalone EventSemaphore sequencer instructions
    placed immediately before (same engine => same stream order)."""
    from bass_rust import SyncInfo
    n = 0
    for f in nc.m.functions:
        for blk in f.blocks:
            new = []
            for inst in blk.instructions:
                si = inst.sync_info
                if si is not None and len(si.on_wait) > 1:
                    for w in si.on_wait[:-1]:
                        n += 1
                        new.append(mybir.InstEventSemaphore(
                            name=f"evw-{n}", engine=inst.engine,
                            sync_info=SyncInfo(on_wait=[w], on_update=[]),
                        ))
                    inst.sync_info = SyncInfo(on_wait=[si.on_wait[-1]],
                                              on_update=list(si.on_update))
                new.append(inst)
            try:
                blk.instructions = new
            except Exception:
                blk.instructions[:] = new
    return n


def _host_pack(x2, kflat, lens_s, starts_s, P, off):
    """Gather x / kernel into the packed padded-ELL slabs, per core."""
    xkell = [np.empty(int(off[-1]), np.float32) for _ in range(NCORES)]
    for sb in range(NSB):
        Ps = int(P[sb])
        st = starts_s[sb * SBC:(sb + 1) * SBC].reshape(G, CPB)
        ln = lens_s[sb * SBC:(sb + 1) * SBC].reshape(G, CPB)
        pr = np.arange(Ps)
        msk = pr < ln[..., None]                        # [G,128,Ps]
        idx = np.where(msk, st[..., None] + pr, 0)
        kslab = ((kflat[idx] * msk).astype(np.float32)
                 .transpose(1, 0, 2).reshape(128, G * Ps))
        gx = x2[:, idx]                                 # [B,G,128,Ps]
        for c in range(NCORES):
            xs = (gx[c * BPC:(c + 1) * BPC]
                  .transpose(2, 0, 1, 3).reshape(128, BPC * G * Ps))
            xkell[c][int(off[sb]):int(off[sb + 1])] = (
                np.concatenate([xs, kslab], axis=1).ravel())
    return xkell


def kernel(x, kernel, bias, mask_row, mask_col, _trace=False):
    x = np.asarray(x, np.float32)
    kflat = np.asarray(kernel, np.float32).reshape(E)
    bias = np.asarray(bias, np.float32)
    mask_col = np.asarray(mask_col)
    x2 = np.ascontiguousarray(x.reshape(B, E))

    perm, lens_s, starts_s, P = _plan(mask_col)
    nc, off = _build_program(P)
    _split_multi_waits(nc)
    xkell = _host_pack(x2, kflat, lens_s, starts_s, P, off)

    in_maps = [{"xkell": xkell[c]} for c in range(NCORES)]
    res = run_bass_kernel_spmd(
        nc, in_maps, core_ids=list(range(NCORES)), trace=_trace)

    out_full = np.zeros((B, NOUT), np.float32)
    for c in range(NCORES):
        arr = np.stack([res.results[c][f"oseg{sb:02d}"]
                        for sb in range(NSB)]).reshape(NSB, 128, BPC, G)
        vals = arr.transpose(2, 0, 3, 1).reshape(BPC, NSB * SBC)[:, NPAD:]
        out_full[c * BPC:(c + 1) * BPC, perm] = vals

    out = out_full[:, :, None] + bias[None, :, :]
    if _trace:
        return out, res
    return out


# revision 11
# speedup vs baseline: 1.0157x; 1.0157x over previous
"""Trainium2 Bass kernel for LocallyDirected1D (gnn_message_passing).

Computation: out[b, j] = sum_{e in [o[j], o[j+1])} x[b, e] * k[e]  (+ bias[j])
where o = CSR offsets of the sorted mask_col, mask_row == arange(E).

Strategy:
  * Data-parallel over batch: core c handles batch rows [8c, 8c+8).
  * Host builds a padded-ELL layout: output columns sorted by segment
    length, grouped into superblocks of 1024 columns (8 groups x 128
    partitions) padded to the superblock max length P.  Padding slots of
    the weight array are zero, so the gathered x can hold garbage there.
    The x slab and the k slab of a superblock are packed into ONE
    contiguous HBM region so a single DMA (= a single semaphore wait)
    brings in both.
  * Device streams the dense slabs: y = x*k on VectorE, segment sums via
    tensor_reduce over the innermost (padded-length) axis, results DMAed
    back and un-permuted on host.
"""

import numpy as np

import concourse.bass as bass
import concourse.mybir as mybir
from concourse.tile import TileContext
from concourse.bass_utils import run_bass_kernel_spmd

B = 64
E = 1_000_000
NOUT = 20_000
NCORES = 8
BPC = B // NCORES          # batch rows per core
CPB = 128                  # columns per block (partition dim)
G = 8                      # blocks per superblock
SBC = CPB * G              # 1024 columns per superblock
NSB = (NOUT + SBC - 1) // SBC   # 20 superblocks
NPAD = NSB * SBC - NOUT    # dummy (zero-length) columns, placed first
ROWW = BPC * G + G         # free elems per partition per unit P (x then k)

F32 = mybir.dt.float32


def _plan(mask_col: np.ndarray):
    """CSR offsets -> length-sorted padded-ELL plan."""
    o = np.searchsorted(mask_col, np.arange(NOUT + 1)).astype(np.int64)
    lens = np.diff(o).astype(np.int64)
    perm = np.argsort(lens, kind="stable").astype(np.int64)
    lens_s = np.concatenate([np.zeros(NPAD, np.int64), lens[perm]])
    starts_s = np.concatenate([np.zeros(NPAD, np.int64), o[:-1][perm]])
    P = lens_s.reshape(NSB, SBC).max(axis=1)
    P = np.maximum(P, 1).astype(np.int64)
    return perm, lens_s, starts_s, P


def _build_program(P, gp_mod=3, gp_keep=2):
    """gp_mod/gp_keep: superblocks with sb % gp_mod < gp_keep run their
    multiply on GPSIMD (~2x slower per element than VectorE but fully
    concurrent); the rest multiply on VectorE.  All reduces are VectorE
    (the only engine with free-axis tensor_reduce)."""
    nc = bass.Bass()
    off = np.concatenate([[0], np.cumsum(128 * ROWW * P)]).astype(np.int64)
    xk_d = nc.dram_tensor("xkell", [int(off[-1])], F32, kind="ExternalInput")
    # one output tensor per superblock: a single shared output tensor makes
    # Tile serialize the store DMAs (tensor-granularity WAW), which puts a
    # second sync-wait on each store — walrus allows only one per instruction
    o_ds = [
        nc.dram_tensor(f"oseg{sb:02d}", [128 * BPC * G], F32,
                       kind="ExternalOutput")
        for sb in range(NSB)
    ]

    with TileContext(nc) as tc:
        with (
            tc.tile_pool(name="xp", bufs=3) as xp,
            tc.tile_pool(name="op", bufs=NSB) as op_,
        ):
            for sb in range(NSB):
                Ps = int(P[sb])
                QF = G * Ps            # free elems per batch row (and k width)
                XF = BPC * QF          # x portion width
                t = xp.tile([128, ROWW * Ps], F32, tag="x")
                nc.sync.dma_start(
                    t[:],
                    xk_d[int(off[sb]):int(off[sb + 1])].rearrange(
                        "(j f) -> j f", j=128),
                )
                eng = nc.gpsimd if (sb % gp_mod) < gp_keep else nc.vector
                xv = t[:, 0:XF].rearrange("j (b q) -> j b q", b=BPC)
                kv = (t[:, XF:XF + QF].unsqueeze(1)
                      .broadcast_to([128, BPC, QF]))
                eng.tensor_tensor(xv, xv, kv, mybir.AluOpType.mult)
                ot = op_.tile([128, BPC * G], F32, tag="o")
                nc.vector.tensor_reduce(
                    ot[:],
                    t[:, 0:XF].rearrange("j (q p) -> j q p", p=Ps),
                    axis=mybir.AxisListType.X,
                    op=mybir.AluOpType.add,
                )
                nc.sync.dma_start(
                    o_ds[sb][:].rearrange("(j f) -> j f", j=128),
                    ot[:],
                )
    return nc, off


def _split_multi_waits(nc):
    """walrus allows at most one sync-wait per engine instruction; hoist
    extra waits into standalone EventSemaphore sequencer instructions
    placed immediately before (same engine => same stream order)."""
    from bass_rust import SyncInfo
    n = 0
    for f in nc.m.functions:
        for blk in f.blocks:
            new = []
            for inst in blk.instructions:
                si = inst.sync_info
                if si is not None and len(si.on_wait) > 1:
                    for w in si.on_wait[:-1]:
                        n += 1
                        new.append(mybir.InstEventSemaphore(
                            name=f"evw-{n}", engine=inst.engine,
                            sync_info=SyncInfo(on_wait=[w], on_update=[]),
                        ))
                    inst.sync_info = SyncInfo(on_wait=[si.on_wait[-1]],
                                              on_update=list(si.on_update))
                new.append(inst)
            try:
                blk.instructions = new
            except Exception:
                blk.instructions[:] = new
    return n


def _host_pack(x2, kflat, lens_s, starts_s, P, off):
    """Gather x / kernel into the packed padded-ELL slabs, per core."""
    xkell = [np.empty(int(off[-1]), np.float32) for _ in range(NCORES)]
    for sb in range(NSB):
        Ps = int(P[sb])
        st = starts_s[sb * SBC:(sb + 1) * SBC].reshape(G, CPB)
        ln = lens_s[sb * SBC:(sb + 1) * SBC].reshape(G, CPB)
        pr = np.arange(Ps)
        msk = pr < ln[..., None]                        # [G,128,Ps]
        idx = np.where(msk, st[..., None] + pr, 0)
        kslab = ((kflat[idx] * msk).astype(np.float32)
                 .transpose(1, 0, 2).reshape(128, G * Ps))
        gx = x2[:, idx]                                 # [B,G,128,Ps]
        for c in range(NCORES):
            xs = (gx[c * BPC:(c + 1) * BPC]
                  .transpose(2, 0, 1, 3).reshape(128, BPC * G * Ps))
            xkell[c][int(off[sb]):int(off[sb + 1])] = (
                np.concatenate([xs, kslab], axis=1).ravel())
    return xkell


def kernel(x, kernel, bias, mask_row, mask_col, _trace=False):
    x = np.asarray(x, np.float32)
    kflat = np.asarray(kernel, np.float32).reshape(E)
    bias = np.asarray(bias, np.float32)
    mask_col = np.asarray(mask_col)
    x2 = np.ascontiguousarray(x.reshape(B, E))

    perm, lens_s, starts_s, P = _plan(mask_col)
    nc, off = _build_program(P)
    _split_multi_waits(nc)
    xkell = _host_pack(x2, kflat, lens_s, starts_s, P, off)

    in_maps = [{"xkell": xkell[c]} for c in range(NCORES)]
    res = run_bass_kernel_spmd(
        nc, in_maps, core_ids=list(range(NCORES)), trace=_trace)

    out_full = np.zeros((B, NOUT), np.float32)
    for c in range(NCORES):
        arr = np.stack([res.results[c][f"oseg{sb:02d}"]
                        for sb in range(NSB)]).reshape(NSB, 128, BPC, G)
        vals = arr.transpose(2, 0, 3, 1).reshape(BPC, NSB * SBC)[:, NPAD:]
        out_full[c * BPC:(c + 1) * BPC, perm] = vals

    out = out_full[:, :, None] + bias[None, :, :]
    if _trace:
        return out, res
    return out
